# revision 31
# baseline (speedup 1.0000x reference)
"""3x3 median filter (reflect padding) on Trainium2, 8-core data parallel.

Input  x: (4, 3, 1024, 1024) float32
Output  : (4, 3, 1024, 1024) float32  (Kornia MedianBlur semantics)

Strategy: 5 hand-written custom DVE uop programs, each running in 2x_1p
perf mode and processing one pixel per cycle (the per-NEFF DVE table
mechanism; rows 17..21).  All tensors use a paired layout [P, 2N] fp16
where positions (2i, 2i+1) belong to pixel i, so each 32-bit SBUF port
read delivers a whole pixel pair and the 8-stage datapath computes a
full network stage per cycle:

  MEDLH:  [T0|T1], [T1|T2]       -> [lo|hi]    column min3 / max3
  MEDMID: [T0|T1], [T1|T2]       -> [mid|mid]  column med3
  MEDAC:  z2, z2 shifted 1 pixel -> [A|C]      sliding max3(lo) / min3(hi)
                                               (swap-flop history)
  MEDB:   zm, zm shifted 1 pixel -> [B|B]      sliding med3(mid)
  MEDFIN: [A|C], [B|B]           -> [out|out]  final med3

This is 5 DVE cycles/pixel vs ~8 for the best tensor_tensor network
(15 ops at 2 elem/cycle), and frees PE/ACT entirely.  The host
interleaves rows into PALL[i, r, 2j] = xp[r, j], [2j+1] = xp[r+1, j];
P01 and P12 are two row-shifted DMA views of it.  Sliding-op history
garbage and image-junction junk land on reflect-pad columns which are
never stored.  Output is stored as pairs and de-interleaved on host.
"""

import sys

sys.path.insert(0, "/opt/trn_rl_repo")

import numpy as np

B, C, H, W = 4, 3, 1024, 1024
NIMG = B * C            # 12
NCORES = 8
RPC = H // NCORES       # 128 output rows per core
WP = W + 2              # 1026 padded width
W2 = 2 * WP             # 2052 paired positions per image row
CHUNKS = (1, 1, 1, 2, 3, 3)   # images per chunk AFTER the two half-image
                              # head chunks covering image 0: single-image
                              # chunks early so each chunk's load barrier is
                              # small and compute can start at ~11.5us
GMAX = max(CHUNKS)
P = 128

_PROGRAM = None
LAST_RESULT = None

# --------------------------------------------------------------------------- #
# Custom DVE uop programs (2x_1p mode: per cycle SRC_0=src0[2i],
# SRC_0_HI=src0[2i+1], SRC_1=src1[2i], SRC_1_HI=src1[2i+1];
# WR0_LO -> dst[2i], WR0_HI -> dst[2i+1]).
# --------------------------------------------------------------------------- #

_VER_TAG = "V1"
_ROW_BASE = 17  # stock custom-DVE ops occupy rows 1..16; field allows < 32


def _build_ops():
    from concourse.dve_uop import (
        ENABLE,
        AluInp,
        AluOp,
        DelayInp,
        DveOpSpec,
        InpSel,
        OutPath,
        OutSel,
        Trigger,
        UopConfig,
    )

    MN, MX, BYP = AluOp.MIN, AluOp.MAX, AluOp.BYPASS
    PA, SW = AluInp.PREV_ALU_OUT, AluInp.CURR_SWAP_OUT
    D = [AluInp.PREV_DELAY_0, AluInp.PREV_DELAY_1, AluInp.PREV_DELAY_2,
         AluInp.PREV_DELAY_3, AluInp.PREV_DELAY_4, AluInp.PREV_DELAY_5]

    def base_uop():
        u = UopConfig()
        u.require_inp0 = ENABLE
        u.require_inp1 = ENABLE
        u.trigger = (Trigger.SRC_TENSOR_DONE, Trigger.NONE, Trigger.NONE)
        u.next_uop = (0, 0, 0)
        return u

    def fallback_1x():
        # Only reached if the RTL declines 2x mode; copies src0 through so
        # the run completes (visibly wrong) instead of hanging.
        u = base_uop()
        u.enable_input(InpSel.SRC_0, 0)
        u.enable_input(InpSel.SRC_1, 1)
        for k in range(8):
            u.datapath_config[k].pass_through_alu()
            if k == 0:
                u.datapath_config[k].enable_delay_from_src(DelayInp.PREV_ALU_OUT, 0)
            else:
                u.datapath_config[k].pass_through_delay(0)
        u.enable_output(OutSel.ALU_OUT, OutPath.WR0_LO)
        return u

    def medlh_2x():
        u = base_uop()
        u.enable_input(InpSel.SRC_0, 0)      # T0
        u.enable_input(InpSel.SRC_0_HI, 1)   # T1 -> d0
        u.enable_input(InpSel.SRC_1_HI, 2)   # T2 -> d1
        dp = u.datapath_config
        dp[0].enable_alu(MN, PA, D[0])                          # m
        dp[0].enable_delay_from_src(DelayInp.PREV_ALU_OUT, 2)   # T0 -> d2
        dp[0].pass_through_delay(0, 1)
        dp[1].enable_alu(MX, D[2], D[0])                        # M
        dp[1].enable_delay_from_src(DelayInp.PREV_ALU_OUT, 3)   # m -> d3
        dp[1].pass_through_delay(1)
        dp[2].enable_alu(MN, D[3], D[1])                        # lo
        dp[2].enable_delay_from_src(DelayInp.PREV_ALU_OUT, 4)   # M -> d4
        dp[2].pass_through_delay(1)
        dp[3].enable_alu(MX, D[4], D[1])                        # hi
        dp[3].enable_delay_from_src(DelayInp.PREV_ALU_OUT, 5)   # lo -> d5
        for k in range(4, 8):
            dp[k].pass_through_alu()
            dp[k].pass_through_delay(5)
        u.enable_output(OutSel.DELAY_5, OutPath.WR0_LO)   # lo
        u.enable_output(OutSel.ALU_OUT, OutPath.WR0_HI)   # hi
        return u

    def medmid_2x():
        u = base_uop()
        u.enable_input(InpSel.SRC_0, 0)
        u.enable_input(InpSel.SRC_0_HI, 1)
        u.enable_input(InpSel.SRC_1_HI, 2)
        dp = u.datapath_config
        dp[0].enable_alu(MN, PA, D[0])                          # m
        dp[0].enable_delay_from_src(DelayInp.PREV_ALU_OUT, 2)   # T0 -> d2
        dp[0].pass_through_delay(0, 1)
        dp[1].enable_alu(MX, D[2], D[0])                        # M
        dp[1].enable_delay_from_src(DelayInp.PREV_ALU_OUT, 3)   # m -> d3
        dp[1].pass_through_delay(1)
        dp[2].enable_alu(MN, PA, D[1])                          # u2 = min(M,T2)
        dp[2].pass_through_delay(3)
        dp[3].enable_alu(MX, PA, D[3])                          # mid = max(u2,m)
        for k in range(4, 8):
            dp[k].pass_through_alu()
        u.enable_output(OutSel.ALU_OUT, OutPath.WR0_LO)
        u.enable_output(OutSel.ALU_OUT, OutPath.WR0_HI)
        return u

    def medac_2x():
        u = base_uop()
        u.enable_input(InpSel.SRC_0, 0)      # lo_j
        u.enable_input(InpSel.SRC_1, 1)      # lo_{j+1} -> d0
        u.enable_input(InpSel.SRC_0_HI, 2)   # hi_j     -> d1
        u.enable_input(InpSel.SRC_1_HI, 3)   # hi_{j+1} -> d2
        dp = u.datapath_config
        dp[0].enable_alu(MX, PA, D[0])       # pa_j
        dp[0].pass_through_delay(0, 1, 2)
        dp[1].enable_alu(BYP, SW, PA)        # emit pa_{j-1}; latch pa_j
        dp[1].swap_enable = ENABLE
        dp[1].pass_through_delay(0, 1, 2)
        dp[2].enable_alu(MX, PA, D[0])       # A_j = max(pa_{j-1}, lo_{j+1})
        dp[2].pass_through_delay(1, 2)
        dp[3].enable_alu(MN, D[1], D[2])     # pc_j
        dp[3].enable_delay_from_src(DelayInp.PREV_ALU_OUT, 3)   # A -> d3
        dp[3].pass_through_delay(2)
        dp[4].enable_alu(BYP, SW, PA)        # emit pc_{j-1}; latch pc_j
        dp[4].swap_enable = ENABLE
        dp[4].pass_through_delay(2, 3)
        dp[5].enable_alu(MN, PA, D[2])       # C_j = min(pc_{j-1}, hi_{j+1})
        dp[5].pass_through_delay(3)
        for k in (6, 7):
            dp[k].pass_through_alu()
            dp[k].pass_through_delay(3)
        u.enable_output(OutSel.DELAY_3, OutPath.WR0_LO)   # A
        u.enable_output(OutSel.ALU_OUT, OutPath.WR0_HI)   # C
        return u

    def medb_2x():
        u = base_uop()
        u.enable_input(InpSel.SRC_0, 0)      # mid_j
        u.enable_input(InpSel.SRC_1, 1)      # mid_{j+1} -> d0
        dp = u.datapath_config
        dp[0].enable_alu(MN, PA, D[0])                          # pm_j
        dp[0].enable_delay_from_src(DelayInp.PREV_ALU_OUT, 1)   # mid_j -> d1
        dp[0].pass_through_delay(0)
        dp[1].enable_alu(BYP, SW, PA)        # emit pm_{j-1}; latch pm_j
        dp[1].swap_enable = ENABLE
        dp[1].pass_through_delay(0, 1)
        dp[2].enable_alu(MX, D[1], D[0])                        # pM_j
        dp[2].enable_delay_from_src(DelayInp.PREV_ALU_OUT, 3)   # pm_{j-1} -> d3
        dp[2].pass_through_delay(0)
        dp[3].enable_alu(BYP, SW, PA)        # emit pM_{j-1}; latch pM_j
        dp[3].swap_enable = ENABLE
        dp[3].pass_through_delay(0, 3)
        dp[4].enable_alu(MN, PA, D[0])       # t = min(pM_{j-1}, mid_{j+1})
        dp[4].pass_through_delay(3)
        dp[5].enable_alu(MX, PA, D[3])       # B_j = max(t, pm_{j-1})
        for k in (6, 7):
            dp[k].pass_through_alu()
        u.enable_output(OutSel.ALU_OUT, OutPath.WR0_LO)
        u.enable_output(OutSel.ALU_OUT, OutPath.WR0_HI)
        return u

    def medfin_2x():
        u = base_uop()
        u.enable_input(InpSel.SRC_0, 0)      # A
        u.enable_input(InpSel.SRC_1, 1)      # B -> d0
        u.enable_input(InpSel.SRC_0_HI, 2)   # C -> d1
        dp = u.datapath_config
        dp[0].enable_alu(MN, PA, D[0])                          # m1
        dp[0].enable_delay_from_src(DelayInp.PREV_ALU_OUT, 2)   # A -> d2
        dp[0].pass_through_delay(0, 1)
        dp[1].enable_alu(MX, D[2], D[0])                        # M1
        dp[1].enable_delay_from_src(DelayInp.PREV_ALU_OUT, 3)   # m1 -> d3
        dp[1].pass_through_delay(1)
        dp[2].enable_alu(MN, PA, D[1])                          # t3 = min(M1,C)
        dp[2].pass_through_delay(3)
        dp[3].enable_alu(MX, PA, D[3])                          # out
        for k in range(4, 8):
            dp[k].pass_through_alu()
        u.enable_output(OutSel.ALU_OUT, OutPath.WR0_LO)
        u.enable_output(OutSel.ALU_OUT, OutPath.WR0_HI)
        return u

    builders = {
        "MEDLH": medlh_2x,
        "MEDMID": medmid_2x,
        "MEDAC": medac_2x,
        "MEDB": medb_2x,
        "MEDFIN": medfin_2x,
    }
    ops = {}
    for i, (nm, fn) in enumerate(builders.items()):
        name = f"{nm}_{_VER_TAG}"
        spec = DveOpSpec(
            name=name,
            opcode=_ROW_BASE + i,
            uops=[fallback_1x()],
            uops_2x=[fn()],
            rd1_en=True,
            perf_max=1,
        )
        spec.validate("v3")
        ops[name] = spec
    return ops


class _NullSpec:
    # CoreSim-only surface; the HW path never evaluates it.
    accum = None

    @staticmethod
    def reference(in0, in1, c0, c1, c2):
        return in0


class _PairedDveOp:
    """Duck-typed dve_ops.DveOp: dve_table_for_ops only uses .name/.compile."""

    subdim = False
    perf_en = {}
    spec = _NullSpec()

    def __init__(self, name, opspec):
        self.name = name
        self.row = opspec.opcode
        self._opspec = opspec

    def compile(self, ver):
        assert ver == "v3", ver
        return self._opspec


_OPS = None


def _register_ops():
    global _OPS
    if _OPS is not None:
        return _OPS
    from concourse import dve_ops

    _OPS = {}
    for name, spec in _build_ops().items():
        op = _PairedDveOp(name, spec)
        if name not in dve_ops._SUB_OPCODE_FOR_NAME:
            dve_ops.OPS.append(op)
            dve_ops._SUB_OPCODE_FOR_NAME[name] = op.row
            dve_ops.CUSTOM_DVE_SPECS[name] = op.spec
        _OPS[name.rsplit("_", 1)[0]] = op
    return _OPS


def _emit(vec, op, out, in0, in1):
    """Emit one paired custom-DVE instruction with perf_max=1 (2x slot)."""
    from concourse import bass_isa, mybir

    nc = vec.bass
    if op.name not in nc.m.ant_custom_dve_ops:
        nc.m.ant_custom_dve_ops = sorted({*nc.m.ant_custom_dve_ops, op.name})
    shape = bass_isa.CustomDveShape.TTSS
    isa_opcode = nc.isa.Opcode[
        f"NEURON_ISA_TPB_OPCODE_CUSTOM_DVE_ANT_{shape.slot()}"
    ].value
    zero = mybir.ImmediateValue(dtype=mybir.dt.float32, value=0.0)
    return vec.add_instruction(
        bass_isa.InstCustomDveAnt(
            name=nc.get_next_instruction_name(),
            op_name=op.name,
            rd1_en=True,
            subdim=0,
            imm2=0.0,
            shape=shape,
            row=op.row,
            isa_opcode=isa_opcode,
            perf_max=1,
            ins=[
                vec.lower_ap(in0, for_isa=True),
                vec.lower_ap(in1, for_isa=True),
                zero,
                zero,
            ],
            outs=[vec.lower_ap(out, for_isa=True)],
        )
    )


# --------------------------------------------------------------------------- #
# Program
# --------------------------------------------------------------------------- #

def _build_program():
    import concourse.bacc as bacc
    import concourse.tile as tile
    import concourse.mybir as mybir
    from contextlib import ExitStack

    ops = _register_ops()
    f16 = mybir.dt.float16

    nc = bacc.Bacc("TRN2", target_bir_lowering=False, debug=False,
                   num_devices=NCORES)
    pall = nc.dram_tensor("pall", [NIMG, P + 1, W2], f16,
                          kind="ExternalInput").ap()
    # image 0 pre-split into contiguous overlapping halves (strided loads of
    # pall run at less than half queue bandwidth; these stream at full rate)
    p0l = nc.dram_tensor("p0l", [P + 1, 1030], f16, kind="ExternalInput").ap()
    p0r = nc.dram_tensor("p0r", [P + 1, 1028], f16, kind="ExternalInput").ap()
    y = nc.dram_tensor("y", [NIMG, P, 2 * W], f16, kind="ExternalOutput").ap()

    with tile.TileContext(nc) as tc, ExitStack() as ctx:
        iopool = ctx.enter_context(tc.tile_pool(name="io", bufs=2))
        mpool = ctx.enter_context(tc.tile_pool(name="mid", bufs=1))
        # zout double-buffered: chunk c+1's MEDFIN must not wait for chunk
        # c's stores to drain
        opool = ctx.enter_context(tc.tile_pool(name="out", bufs=2))
        NB = GMAX * W2
        # DMA queue assignment (queue ORDER within an engine is Tile's call;
        # only the engine choice is ours).  Loads rotate strictly across the
        # three queues in emission order so every queue carries 1/3 of the
        # input stream; stores rotate independently (they are not
        # latency-critical -- zout is double-buffered).
        import itertools
        qeng = {"scalar": nc.scalar, "sync": nc.sync, "gpsimd": nc.gpsimd}
        _lq = itertools.cycle(["scalar", "sync", "gpsimd"])
        _sq = itertools.cycle(["sync", "gpsimd", "scalar"])

        def dq(dst, src, kb):
            qeng[next(_lq)].dma_start(dst, src)

        def dqs(dst, src, kb):
            qeng[next(_sq)].dma_start(dst, src)

        # --- image 0 as two independent half-image chunks --------------- #
        # A: padded cols 0..514, valid outputs cols 1..512
        # B: padded cols 512..1025, valid outputs cols 513..1024
        for half in range(2):
            src = (p0l, p0r)[half]
            wpos = (1030, 1028)[half]
            P01 = iopool.tile([P, NB], f16, tag="P01", name="P01")
            P12 = iopool.tile([P, NB], f16, tag="P12", name="P12")
            z2 = mpool.tile([P, NB], f16, tag="z2", name="z2")
            zm = mpool.tile([P, NB], f16, tag="zm", name="zm")
            zac = mpool.tile([P, NB], f16, tag="zac", name="zac")
            zb = mpool.tile([P, NB], f16, tag="zb", name="zb")
            zout = opool.tile([P, NB], f16, tag="zout", name="zout")
            dq(P01[:, 0:wpos], src[0:P], 0.26 * wpos)
            dq(P12[:, 0:wpos], src[1:P + 1], 0.26 * wpos)
            _emit(nc.vector, ops["MEDLH"], z2[:, 0:wpos], P01[:, 0:wpos],
                  P12[:, 0:wpos])
            _emit(nc.vector, ops["MEDMID"], zm[:, 0:wpos], P01[:, 0:wpos],
                  P12[:, 0:wpos])
            _emit(nc.vector, ops["MEDAC"], zac[:, 0:wpos - 2],
                  z2[:, 0:wpos - 2], z2[:, 2:wpos])
            _emit(nc.vector, ops["MEDB"], zb[:, 0:wpos - 2],
                  zm[:, 0:wpos - 2], zm[:, 2:wpos])
            _emit(nc.vector, ops["MEDFIN"], zout[:, 0:wpos - 2],
                  zac[:, 0:wpos - 2], zb[:, 0:wpos - 2])
            dqs(y[0, :, half * 1024:half * 1024 + 1024], zout[:, 2:1026], 256)

        i0 = 1
        for ci, G in enumerate(CHUNKS):
            N2 = G * W2
            P01 = iopool.tile([P, NB], f16, tag="P01", name="P01")
            P12 = iopool.tile([P, NB], f16, tag="P12", name="P12")
            z2 = mpool.tile([P, NB], f16, tag="z2", name="z2")
            zm = mpool.tile([P, NB], f16, tag="zm", name="zm")
            zac = mpool.tile([P, NB], f16, tag="zac", name="zac")
            zb = mpool.tile([P, NB], f16, tag="zb", name="zb")
            zout = opool.tile([P, NB], f16, tag="zout", name="zout")

            qs = (nc.gpsimd, nc.scalar, nc.sync)

            def run_ops(a, b):
                """Run the 5-op pipeline on positions [a, b) of this chunk's
                tiles.  Sliding ops emit the window centred at each pixel;
                their first pixel (stale swap flop) is garbage."""
                _emit(nc.vector, ops["MEDLH"], z2[:, a:b], P01[:, a:b],
                      P12[:, a:b])
                _emit(nc.vector, ops["MEDMID"], zm[:, a:b], P01[:, a:b],
                      P12[:, a:b])
                _emit(nc.vector, ops["MEDAC"], zac[:, a:b - 2], z2[:, a:b - 2],
                      z2[:, a + 2:b])
                _emit(nc.vector, ops["MEDB"], zb[:, a:b - 2], zm[:, a:b - 2],
                      zm[:, a + 2:b])
                _emit(nc.vector, ops["MEDFIN"], zout[:, a:b - 2],
                      zac[:, a:b - 2], zb[:, a:b - 2])

            for g in range(G):
                s = slice(g * W2, (g + 1) * W2)
                dq(P01[:, s], pall[i0 + g, 0:P, :], 525)
                dq(P12[:, s], pall[i0 + g, 1:P + 1, :], 525)
            if ci == len(CHUNKS) - 1:
                # tail chunk: column+sliding ops whole; FIN per image
                # (stateless, so any split is safe) with an immediate
                # store; the last image's FIN runs in thirds so the
                # final transfer is short.
                _emit(nc.vector, ops["MEDLH"], z2[:, 0:N2], P01[:, 0:N2],
                      P12[:, 0:N2])
                _emit(nc.vector, ops["MEDMID"], zm[:, 0:N2], P01[:, 0:N2],
                      P12[:, 0:N2])
                n1 = N2 - 2
                _emit(nc.vector, ops["MEDAC"], zac[:, 0:n1], z2[:, 0:n1],
                      z2[:, 2:N2])
                _emit(nc.vector, ops["MEDB"], zb[:, 0:n1], zm[:, 0:n1],
                      zm[:, 2:N2])
                for g in range(G - 1):
                    a = g * W2 + 2
                    _emit(nc.vector, ops["MEDFIN"], zout[:, a:a + 2048],
                          zac[:, a:a + 2048], zb[:, a:a + 2048])
                    dqs(y[i0 + g], zout[:, a:a + 2048], 512)
                b0 = (G - 1) * W2
                cuts = (b0 + 2, b0 + 686, b0 + 1370, b0 + 2050)
                for t in range(3):
                    a, b = cuts[t], cuts[t + 1]
                    _emit(nc.vector, ops["MEDFIN"], zout[:, a:b],
                          zac[:, a:b], zb[:, a:b])
                    (nc.scalar, nc.sync, nc.scalar)[t].dma_start(
                        y[i0 + G - 1, :, a - b0 - 2:b - b0 - 2],
                        zout[:, a:b])
                i0 += G
                continue
            run_ops(0, N2)

            for g in range(G):
                # valid pixels: image cols 1..1024 -> positions base..base+2048
                base = (g * WP + 1) * 2
                dqs(y[i0 + g], zout[:, base:base + 2 * W], 512)
            i0 += G

    nc.compile()
    return nc


def _get_program():
    global _PROGRAM
    if _PROGRAM is None:
        _PROGRAM = _build_program()
    return _PROGRAM


def kernel(x):
    global LAST_RESULT
    from concourse.bass_utils import run_bass_kernel_spmd
    import os

    x16 = np.asarray(x).astype(np.float16).reshape(NIMG, H, W)
    xp = np.pad(x16, ((0, 0), (1, 1), (1, 1)), mode="reflect")
    in_maps = []
    for k in range(NCORES):
        slab = xp[:, RPC * k: RPC * k + RPC + 2, :]  # [NIMG, 130, 1026]
        pall = np.empty((NIMG, P + 1, W2), np.float16)
        pall[:, :, 0::2] = slab[:, 0:P + 1, :]
        pall[:, :, 1::2] = slab[:, 1:P + 2, :]
        in_maps.append({
            "pall": pall,
            "p0l": np.ascontiguousarray(pall[0, :, 0:1030]),
            "p0r": np.ascontiguousarray(pall[0, :, 1024:W2]),
        })

    nc = _get_program()
    trace = bool(int(os.environ.get("MEDIAN_TRACE", "0")))
    res = run_bass_kernel_spmd(nc, in_maps, list(range(NCORES)), trace=trace)
    LAST_RESULT = res
    out = np.concatenate(
        [res.results[k]["y"][:, :, 0::2] for k in range(NCORES)], axis=1)
    return out.reshape(B, C, H, W).astype(np.float32)
